# revision 27
# baseline (speedup 1.0000x reference)
"""GCN layer (SpMM + Linear) on 8 Trainium2 NeuronCores.

out[i] = (sum_{e: row[e]==i} val[e] * X[col[e]]) @ W.T + b

Strategy v2:
- Destinations sharded across 8 cores (12500 rows each, padded to
  12544 = 98 supers of 128 dests).
- Edges partitioned by (dest super, source chunk); 4 source chunks of
  25000 rows so chunk-local indices fit int16.
- X pre-cast fp16 on host. Per group, edge source rows are gathered
  from HBM via dma_gather (SWDGE). Gathers round-robin over the 4
  SWDGE queues (Q7 core pairs) so descriptor generation for the 4
  chunks of a super runs concurrently on 4 core pairs.
- Per-queue packed index planes: queue q's core pair only reads SBUF
  partitions [32q, 32q+32), so idx planes for queue q's groups are
  stored only there (2 copies of the 16-partition wrap), quartering
  the SBUF idx footprint.
- Trailing padded edges get idx=-1: the gather ucode trims trailing
  negatives, skipping whole 128-blocks of descriptor generation.
  Stale msgs data is harmless (one-hot val=0, buffers pre-zeroed).
- Aggregation via one-hot matmul, one-hot stationary:
  psum_h[d, f] += oh[e, d].T @ msgs[e, f], oh built on DVE with all
  fp16 operands (is_equal x mult, scalars from a fp16 meta plane).
- Linear on-chip: h -> fp16, PE-transpose to hT, out = hT.T @ W.T via
  two fp16 matmuls. Bias added on host.
"""

import math
from contextlib import ExitStack

import numpy as np

N_NODES = 100000
N_EDGES = 3200000
D = 256
NCORES = 8
SUPER_W = 128
N_CHUNKS = 4
NQ = 4
MSGS_BUFS = 12

_PROGRAM_CACHE = {}


def _patch_tile_drain():
    """Split end-of-kernel drain waits into 1-sem carrier nops.

    The walrus build in this container rejects TPB_CTRL instructions
    with more than one sync wait ("Too many sync wait commands"); Tile's
    stock _drain_and_barrier puts the whole global clock on one drain.
    """
    import concourse.tile as tile
    from concourse.vector_clock import ScopedClock, VectorClock

    if getattr(tile.TileContext, "_drain_patched", False):
        return

    def _drain_and_barrier(self, tick_clock, wait_clock):
        nc = self.nc
        vc = tick_clock.global_clock
        for p in range(len(vc)):
            if vc[p] > 0:
                sub = VectorClock()
                sub.require_at_least(p, vc[p])
                carrier = nc.sync.nop()
                wait_clock.add_sem_waits(carrier.ins, ScopedClock({None: sub}))
        nc.sync.drain()
        nc.all_engine_barrier()
        assert self.sems is not None
        popped = nc._tile_sem_poison_stack.pop()
        assert popped is self._sem_poison
        nc.clear_and_free_semaphores(list(self.sems.allocated().values()))
        nc.all_engine_barrier()

    tile.TileContext._drain_and_barrier = _drain_and_barrier
    tile.TileContext._drain_patched = True


def _plan(edge_row, edge_col):
    """Static group plan shared by all cores."""
    rows_per_core = N_NODES // NCORES
    n_supers = math.ceil(rows_per_core / SUPER_W)
    chunk_sz = N_NODES // N_CHUNKS

    core = edge_row // rows_per_core
    r_local = edge_row - core * rows_per_core
    sup = r_local // SUPER_W
    chunk = edge_col // chunk_sz
    gid = sup * N_CHUNKS + chunk
    n_groups = n_supers * N_CHUNKS

    counts = np.zeros((NCORES, n_groups), np.int64)
    np.add.at(counts, (core, gid), 1)
    caps = counts.max(axis=0)
    caps = np.maximum(((caps + 127) // 128) * 128, 128)
    return caps, core, r_local, sup, chunk, gid, n_supers, chunk_sz


def _layout(caps, n_supers):
    """Column layouts: per-queue idx planes + global fp16 meta plane.

    idx_off[gi]: column offset of group gi's idx plane within its
    queue's region (queue = gi % NQ).  batch_of[gi]: first global batch
    index of group gi (meta columns are 2 per batch).
    """
    n_groups = len(caps)
    idx_off = np.zeros(n_groups, np.int64)
    qcols = np.zeros(NQ, np.int64)
    batch_of = np.zeros(n_groups, np.int64)
    nb_total = 0
    for gi in range(n_groups):
        q = gi % NQ
        idx_off[gi] = qcols[q]
        qcols[q] += caps[gi] // 16
        batch_of[gi] = nb_total
        nb_total += caps[gi] // 128
    idx_cols = int(qcols.max())
    return idx_off, idx_cols, batch_of, nb_total


def _build_program(caps, n_supers, chunk_sz):
    import concourse.bacc as bacc
    import concourse.mybir as mybir
    import concourse.tile as tile

    fp16 = mybir.dt.float16
    fp32 = mybir.dt.float32
    int16 = mybir.dt.int16
    n_groups = len(caps)
    rows_pad = n_supers * SUPER_W
    idx_off, idx_cols, batch_of, nb_total = _layout(caps, n_supers)
    caps2 = caps.reshape(-1, 2, N_CHUNKS).sum(axis=1)  # per (super-pair, chunk)
    nb_max = int(caps2.max()) // 128

    nc = bacc.Bacc("TRN2", target_bir_lowering=False, num_swdge_queues=NQ,
                   dynamic_dma_scratch_size=16384)
    X16 = nc.dram_tensor("x16", [N_NODES, D], fp16, kind="ExternalInput")
    IDX = nc.dram_tensor("idx", [128, idx_cols], int16, kind="ExternalInput")
    META = nc.dram_tensor("meta", [128, 2 * nb_total], fp32,
                          kind="ExternalInput")
    IOTA = nc.dram_tensor("iota", [128, SUPER_W], fp16, kind="ExternalInput")
    IDENT = nc.dram_tensor("ident", [128, 128], fp32, kind="ExternalInput")
    WT = nc.dram_tensor("wt", [128, 2, D], fp16, kind="ExternalInput")
    OUT = nc.dram_tensor("out", [rows_pad, D], fp32, kind="ExternalOutput")

    with tile.TileContext(nc) as tc, ExitStack() as ctx:
        const_pool = ctx.enter_context(tc.tile_pool(name="const", bufs=1))
        msgs_pool = ctx.enter_context(
            tc.tile_pool(name="msgs", bufs=1))
        o_pool = ctx.enter_context(tc.tile_pool(name="onehot", bufs=1))
        h_pool = ctx.enter_context(tc.tile_pool(name="h", bufs=1))
        ht_pool = ctx.enter_context(tc.tile_pool(name="ht", bufs=1))
        out_pool = ctx.enter_context(tc.tile_pool(name="outp", bufs=1))
        psum_h = ctx.enter_context(
            tc.tile_pool(name="psum_h", bufs=1, space="PSUM"))
        psum_t = ctx.enter_context(
            tc.tile_pool(name="psum_t", bufs=1, space="PSUM"))
        psum_o = ctx.enter_context(
            tc.tile_pool(name="psum_o", bufs=1, space="PSUM"))

        idx_t = const_pool.tile([128, idx_cols], int16)
        nc.sync.dma_start(idx_t[:], IDX[:])
        meta_t = const_pool.tile([128, 2 * nb_total], fp32)
        nc.sync.dma_start(meta_t[:], META[:])
        iota_t = const_pool.tile([128, SUPER_W], fp16)
        nc.sync.dma_start(iota_t[:], IOTA[:])
        ident_t = const_pool.tile([128, 128], fp32)
        nc.sync.dma_start(ident_t[:], IDENT[:])
        wt_t = const_pool.tile([128, 2, D], fp16)
        nc.sync.dma_start(wt_t[:], WT[:])

        for sp in range(n_supers // 2):
            subs = (2 * sp, 2 * sp + 1)
            ph = {}
            first = {}
            for s in subs:
                ph[s] = psum_h.tile([128, D], fp32, tag=f"ph{s % 3}",
                                    name=f"ph_{s}")
                first[s] = True
            for c in range(N_CHUNKS):
                gis = [s * N_CHUNKS + c for s in subs]
                cap2 = int(caps[gis[0]]) + int(caps[gis[1]])
                mt = msgs_pool.tile([128, nb_max, D], fp16,
                                    tag=f"m{(sp * N_CHUNKS + c) % MSGS_BUFS}")
                io = int(idx_off[gis[0]])
                nc.gpsimd.dma_gather(
                    mt[:, 0:cap2 // 128, :],
                    X16[c * chunk_sz:(c + 1) * chunk_sz, :],
                    idx_t[:, io:io + cap2 // 16],
                    cap2,
                    cap2,
                    D,
                    elem_step=D,
                    single_packet=(cap2 <= 1024),
                    queue_num=c,
                )
                joff = 0
                for s, gi in zip(subs, gis):
                    nb = int(caps[gi]) // 128
                    for j in range(nb):
                        b = int(batch_of[gi]) + j
                        oh = o_pool.tile([128, SUPER_W], fp16,
                                         tag=f"o{b % 32}")
                        nc.vector.tensor_scalar(
                            oh[:],
                            iota_t[:],
                            meta_t[:, 2 * b:2 * b + 1],
                            meta_t[:, 2 * b + 1:2 * b + 2],
                            mybir.AluOpType.is_equal,
                            mybir.AluOpType.mult,
                        )
                        last = (c == N_CHUNKS - 1) and (j == nb - 1)
                        nc.tensor.matmul(ph[s][:], oh[:], mt[:, joff + j, :],
                                         start=first[s], stop=last)
                        first[s] = False
                    joff += nb

            for s in subs:
                phs = ph[s]
                h_sb = h_pool.tile([128, D], fp32, tag=f"h{s % 3}")
                nc.scalar.copy(h_sb[:], phs[:])
                pt0 = psum_t.tile([128, 128], fp32, tag="pt0")
                pt1 = psum_t.tile([128, 128], fp32, tag="pt1")
                nc.tensor.transpose(pt0[:], h_sb[:, 0:128], ident_t[:])
                nc.tensor.transpose(pt1[:], h_sb[:, 128:256], ident_t[:])
                ht = ht_pool.tile([128, 2, 128], fp16, tag=f"t{s % 3}")
                nc.vector.tensor_copy(ht[:, 0, :], pt0[:])
                nc.vector.tensor_copy(ht[:, 1, :], pt1[:])
                po = psum_o.tile([128, D], fp32, tag=f"po{s % 2}")
                nc.tensor.matmul(po[:], ht[:, 0, :], wt_t[:, 0, :],
                                 start=True, stop=False)
                nc.tensor.matmul(po[:], ht[:, 1, :], wt_t[:, 1, :],
                                 start=False, stop=True)
                ot = out_pool.tile([128, D], fp32, tag=f"ot{s % 3}")
                nc.scalar.copy(ot[:], po[:])
                nc.sync.dma_start(OUT[s * SUPER_W:(s + 1) * SUPER_W, :], ot[:])
    nc.finalize()
    return nc


def _pack_core(k, caps, core, r_local, sup, chunk, gid, edge_col, edge_val,
               chunk_sz, idx_off, idx_cols, nb_total):
    """Build core k's idx plane [128, idx_cols] and meta [128, 2*nb_total]."""
    n_groups = len(caps)
    sel = np.flatnonzero(core == k)
    g = gid[sel]
    order = np.argsort(g, kind="stable")
    sel = sel[order]
    g = g[order]

    cap_off = np.zeros(n_groups + 1, np.int64)
    np.cumsum(caps, out=cap_off[1:])
    grp_start = np.searchsorted(g, np.arange(n_groups))
    grp_end = np.searchsorted(g, np.arange(n_groups), side="right")
    rank = np.arange(len(g)) - grp_start[g]
    pos = cap_off[g] + rank

    total = int(cap_off[-1])
    lc = np.zeros(total, np.int16)
    rl = np.zeros(total, np.float32)
    vv = np.zeros(total, np.float32)
    lc[pos] = (edge_col[sel] - chunk[sel] * chunk_sz).astype(np.int16)
    rl[pos] = (r_local[sel] - sup[sel] * SUPER_W).astype(np.float32)
    vv[pos] = edge_val[sel].astype(np.float32)

    idx_plane = np.zeros((128, idx_cols), np.int16)
    meta = np.zeros((128, 2 * nb_total), np.float32)
    b = 0
    for gi in range(n_groups):
        a, e = int(cap_off[gi]), int(cap_off[gi + 1])
        cap = e - a
        nb = cap // 128
        q = gi % NQ
        # idx: wrapped in 16 partitions, 2 copies for the queue's pair
        w16 = lc[a:e].reshape(cap // 16, 16).T  # [16, cap/16]
        o = int(idx_off[gi])
        idx_plane[32 * q:32 * q + 16, o:o + cap // 16] = w16
        idx_plane[32 * q + 16:32 * q + 32, o:o + cap // 16] = w16
        # meta: per batch (row, val) fp16 per partition
        meta[:, 2 * b:2 * (b + nb):2] = rl[a:e].reshape(nb, 128).T
        meta[:, 2 * b + 1:2 * (b + nb):2] = vv[a:e].reshape(nb, 128).T
        b += nb
    return idx_plane, meta


def _prepare(X, edge_row, edge_col, edge_val, W):
    X = np.asarray(X)
    edge_row = np.asarray(edge_row)
    edge_col = np.asarray(edge_col)
    edge_val = np.asarray(edge_val)
    W = np.asarray(W)

    caps, core, r_local, sup, chunk, gid, n_supers, chunk_sz = _plan(
        edge_row, edge_col)
    idx_off, idx_cols, batch_of, nb_total = _layout(caps, n_supers)

    key = tuple(caps.tolist())
    if key not in _PROGRAM_CACHE:
        _PROGRAM_CACHE[key] = _build_program(caps, n_supers, chunk_sz)
    nc = _PROGRAM_CACHE[key]

    X16 = np.ascontiguousarray(X.astype(np.float16))
    iota = np.tile(np.arange(SUPER_W, dtype=np.float16), (128, 1))
    ident = np.eye(128, dtype=np.float32)
    wt = np.ascontiguousarray(
        W.T.astype(np.float16).reshape(2, 128, D).transpose(1, 0, 2))

    in_maps = []
    for k in range(NCORES):
        idx_plane, meta = _pack_core(
            k, caps, core, r_local, sup, chunk, gid, edge_col, edge_val,
            chunk_sz, idx_off, idx_cols, nb_total)
        in_maps.append({"x16": X16, "idx": idx_plane, "meta": meta,
                        "iota": iota, "ident": ident, "wt": wt})
    return nc, in_maps


def _gather_out(res, b):
    rows_per_core = N_NODES // NCORES
    out = np.empty((N_NODES, D), np.float32)
    for k in range(NCORES):
        out[k * rows_per_core:(k + 1) * rows_per_core] = \
            res.results[k]["out"][:rows_per_core]
    out += np.asarray(b).astype(np.float32)[None, :]
    return out


def kernel(X, edge_row, edge_col, edge_val, W, b):
    from concourse.bass_utils import run_bass_kernel_spmd

    nc, in_maps = _prepare(X, edge_row, edge_col, edge_val, W)
    res = run_bass_kernel_spmd(nc, in_maps, core_ids=list(range(NCORES)))
    return _gather_out(res, b)


def run_traced(X, edge_row, edge_col, edge_val, W, b):
    """Run with NTFF profiling; returns BassKernelResults."""
    from concourse.bass_utils import run_bass_kernel_spmd

    nc, in_maps = _prepare(X, edge_row, edge_col, edge_val, W)
    return run_bass_kernel_spmd(nc, in_maps, core_ids=list(range(NCORES)),
                                trace=True)


# revision 28
# speedup vs baseline: 1.6712x; 1.6712x over previous
"""GCN layer (SpMM + Linear) on 8 Trainium2 NeuronCores.

out[i] = (sum_{e: row[e]==i} val[e] * X[col[e]]) @ W.T + b

Strategy v2:
- Destinations sharded across 8 cores (12500 rows each, padded to
  12544 = 98 supers of 128 dests).
- Edges partitioned by (dest super, source chunk); 4 source chunks of
  25000 rows so chunk-local indices fit int16.
- X pre-cast fp16 on host. Per group, edge source rows are gathered
  from HBM via dma_gather (SWDGE). Gathers round-robin over the 4
  SWDGE queues (Q7 core pairs) so descriptor generation for the 4
  chunks of a super runs concurrently on 4 core pairs.
- Per-queue packed index planes: queue q's core pair only reads SBUF
  partitions [32q, 32q+32), so idx planes for queue q's groups are
  stored only there (2 copies of the 16-partition wrap), quartering
  the SBUF idx footprint.
- Trailing padded edges get idx=-1: the gather ucode trims trailing
  negatives, skipping whole 128-blocks of descriptor generation.
  Stale msgs data is harmless (one-hot val=0, buffers pre-zeroed).
- Aggregation via one-hot matmul, one-hot stationary:
  psum_h[d, f] += oh[e, d].T @ msgs[e, f], oh built on DVE with all
  fp16 operands (is_equal x mult, scalars from a fp16 meta plane).
- Linear on-chip: h -> fp16, PE-transpose to hT, out = hT.T @ W.T via
  two fp16 matmuls. Bias added on host.
"""

import math
from contextlib import ExitStack

import numpy as np

N_NODES = 100000
N_EDGES = 3200000
D = 256
NCORES = 8
SUPER_W = 128
N_CHUNKS = 4
NQ = 4
MSGS_BUFS = 20

_PROGRAM_CACHE = {}


def _patch_tile_drain():
    """Split end-of-kernel drain waits into 1-sem carrier nops.

    The walrus build in this container rejects TPB_CTRL instructions
    with more than one sync wait ("Too many sync wait commands"); Tile's
    stock _drain_and_barrier puts the whole global clock on one drain.
    """
    import concourse.tile as tile
    from concourse.vector_clock import ScopedClock, VectorClock

    if getattr(tile.TileContext, "_drain_patched", False):
        return

    def _drain_and_barrier(self, tick_clock, wait_clock):
        nc = self.nc
        vc = tick_clock.global_clock
        for p in range(len(vc)):
            if vc[p] > 0:
                sub = VectorClock()
                sub.require_at_least(p, vc[p])
                carrier = nc.sync.nop()
                wait_clock.add_sem_waits(carrier.ins, ScopedClock({None: sub}))
        nc.sync.drain()
        nc.all_engine_barrier()
        assert self.sems is not None
        popped = nc._tile_sem_poison_stack.pop()
        assert popped is self._sem_poison
        nc.clear_and_free_semaphores(list(self.sems.allocated().values()))
        nc.all_engine_barrier()

    tile.TileContext._drain_and_barrier = _drain_and_barrier
    tile.TileContext._drain_patched = True


def _plan(edge_row, edge_col):
    """Static group plan shared by all cores."""
    rows_per_core = N_NODES // NCORES
    n_supers = math.ceil(rows_per_core / SUPER_W)
    chunk_sz = N_NODES // N_CHUNKS

    core = edge_row // rows_per_core
    r_local = edge_row - core * rows_per_core
    sup = r_local // SUPER_W
    chunk = edge_col // chunk_sz
    gid = sup * N_CHUNKS + chunk
    n_groups = n_supers * N_CHUNKS

    counts = np.zeros((NCORES, n_groups), np.int64)
    np.add.at(counts, (core, gid), 1)
    caps = counts.max(axis=0)
    caps = np.maximum(((caps + 127) // 128) * 128, 128)
    return caps, core, r_local, sup, chunk, gid, n_supers, chunk_sz


def _layout(caps, n_supers):
    """Column layouts: per-queue idx planes + global fp16 meta plane.

    idx_off[gi]: column offset of group gi's idx plane within its
    queue's region (queue = gi % NQ).  batch_of[gi]: first global batch
    index of group gi (meta columns are 2 per batch).
    """
    n_groups = len(caps)
    idx_off = np.zeros(n_groups, np.int64)
    qcols = np.zeros(NQ, np.int64)
    batch_of = np.zeros(n_groups, np.int64)
    nb_total = 0
    for gi in range(n_groups):
        q = gi % NQ
        idx_off[gi] = qcols[q]
        qcols[q] += caps[gi] // 16
        batch_of[gi] = nb_total
        nb_total += caps[gi] // 128
    idx_cols = int(qcols.max())
    return idx_off, idx_cols, batch_of, nb_total


def _build_program(caps, n_supers, chunk_sz):
    import concourse.bacc as bacc
    import concourse.mybir as mybir
    import concourse.tile as tile

    fp16 = mybir.dt.float16
    fp32 = mybir.dt.float32
    int16 = mybir.dt.int16
    n_groups = len(caps)
    rows_pad = n_supers * SUPER_W
    idx_off, idx_cols, batch_of, nb_total = _layout(caps, n_supers)
    nb_max = int(caps.max()) // 128

    nc = bacc.Bacc("TRN2", target_bir_lowering=False, num_swdge_queues=NQ,
                   dynamic_dma_scratch_size=65536)
    X16 = nc.dram_tensor("x16", [N_NODES, D], fp16, kind="ExternalInput")
    IDX = nc.dram_tensor("idx", [128, idx_cols], int16, kind="ExternalInput")
    META = nc.dram_tensor("meta", [128, 2 * nb_total], fp32,
                          kind="ExternalInput")
    IOTA = nc.dram_tensor("iota", [128, SUPER_W], fp16, kind="ExternalInput")
    IDENT = nc.dram_tensor("ident", [128, 128], fp32, kind="ExternalInput")
    WT = nc.dram_tensor("wt", [128, 2, D], fp16, kind="ExternalInput")
    OUT = nc.dram_tensor("out", [rows_pad, D], fp32, kind="ExternalOutput")

    with tile.TileContext(nc) as tc, ExitStack() as ctx:
        const_pool = ctx.enter_context(tc.tile_pool(name="const", bufs=1))
        msgs_pool = ctx.enter_context(
            tc.tile_pool(name="msgs", bufs=1))
        o_pool = ctx.enter_context(tc.tile_pool(name="onehot", bufs=1))
        h_pool = ctx.enter_context(tc.tile_pool(name="h", bufs=1))
        ht_pool = ctx.enter_context(tc.tile_pool(name="ht", bufs=1))
        out_pool = ctx.enter_context(tc.tile_pool(name="outp", bufs=1))
        psum_h = ctx.enter_context(
            tc.tile_pool(name="psum_h", bufs=1, space="PSUM"))
        psum_t = ctx.enter_context(
            tc.tile_pool(name="psum_t", bufs=1, space="PSUM"))
        psum_o = ctx.enter_context(
            tc.tile_pool(name="psum_o", bufs=1, space="PSUM"))

        idx_t = const_pool.tile([128, idx_cols], int16)
        nc.sync.dma_start(idx_t[:], IDX[:])
        meta_t = const_pool.tile([128, 2 * nb_total], fp32)
        nc.sync.dma_start(meta_t[:], META[:])
        iota_t = const_pool.tile([128, SUPER_W], fp16)
        nc.sync.dma_start(iota_t[:], IOTA[:])
        ident_t = const_pool.tile([128, 128], fp32)
        nc.sync.dma_start(ident_t[:], IDENT[:])
        wt_t = const_pool.tile([128, 2, D], fp16)
        nc.sync.dma_start(wt_t[:], WT[:])

        for s in range(n_supers):
            ph = psum_h.tile([128, D], fp32, tag=f"ph{s % 3}")
            first = True
            for c in range(N_CHUNKS):
                gi = s * N_CHUNKS + c
                cap = int(caps[gi])
                nb = cap // 128
                mt = msgs_pool.tile([128, nb_max, D], fp16,
                                    tag=f"m{gi % MSGS_BUFS}")
                nc.gpsimd.dma_gather(
                    mt[:, 0:nb, :],
                    X16[c * chunk_sz:(c + 1) * chunk_sz, :],
                    idx_t[:, int(idx_off[gi]):int(idx_off[gi]) + cap // 16],
                    cap,
                    cap,
                    D,
                    elem_step=D,
                    single_packet=(cap <= 1024),
                    queue_num=gi % NQ,
                )
                for j in range(nb):
                    b = int(batch_of[gi]) + j
                    oh = o_pool.tile([128, SUPER_W], fp16,
                                     tag=f"o{b % 32}")
                    nc.vector.tensor_scalar(
                        oh[:],
                        iota_t[:],
                        meta_t[:, 2 * b:2 * b + 1],
                        meta_t[:, 2 * b + 1:2 * b + 2],
                        mybir.AluOpType.is_equal,
                        mybir.AluOpType.mult,
                    )
                    last = (c == N_CHUNKS - 1) and (j == nb - 1)
                    nc.tensor.matmul(ph[:], oh[:], mt[:, j, :],
                                     start=first, stop=last)
                    first = False

            h_sb = h_pool.tile([128, D], fp32, tag=f"h{s % 3}")
            nc.scalar.copy(h_sb[:], ph[:])
            pt0 = psum_t.tile([128, 128], fp32, tag="pt0")
            pt1 = psum_t.tile([128, 128], fp32, tag="pt1")
            nc.tensor.transpose(pt0[:], h_sb[:, 0:128], ident_t[:])
            nc.tensor.transpose(pt1[:], h_sb[:, 128:256], ident_t[:])
            ht = ht_pool.tile([128, 2, 128], fp16, tag=f"t{s % 3}")
            nc.vector.tensor_copy(ht[:, 0, :], pt0[:])
            nc.vector.tensor_copy(ht[:, 1, :], pt1[:])
            po = psum_o.tile([128, D], fp32, tag=f"po{s % 2}")
            nc.tensor.matmul(po[:], ht[:, 0, :], wt_t[:, 0, :],
                             start=True, stop=False)
            nc.tensor.matmul(po[:], ht[:, 1, :], wt_t[:, 1, :],
                             start=False, stop=True)
            ot = out_pool.tile([128, D], fp32, tag=f"ot{s % 3}")
            nc.scalar.copy(ot[:], po[:])
            nc.sync.dma_start(OUT[s * SUPER_W:(s + 1) * SUPER_W, :], ot[:])
    nc.finalize()
    return nc


def _pack_core(k, caps, core, r_local, sup, chunk, gid, edge_col, edge_val,
               chunk_sz, idx_off, idx_cols, nb_total):
    """Build core k's idx plane [128, idx_cols] and meta [128, 2*nb_total]."""
    n_groups = len(caps)
    sel = np.flatnonzero(core == k)
    g = gid[sel]
    order = np.argsort(g, kind="stable")
    sel = sel[order]
    g = g[order]

    cap_off = np.zeros(n_groups + 1, np.int64)
    np.cumsum(caps, out=cap_off[1:])
    grp_start = np.searchsorted(g, np.arange(n_groups))
    grp_end = np.searchsorted(g, np.arange(n_groups), side="right")
    rank = np.arange(len(g)) - grp_start[g]
    pos = cap_off[g] + rank

    total = int(cap_off[-1])
    lc = np.zeros(total, np.int16)
    rl = np.zeros(total, np.float32)
    vv = np.zeros(total, np.float32)
    lc[pos] = (edge_col[sel] - chunk[sel] * chunk_sz).astype(np.int16)
    rl[pos] = (r_local[sel] - sup[sel] * SUPER_W).astype(np.float32)
    vv[pos] = edge_val[sel].astype(np.float32)

    idx_plane = np.zeros((128, idx_cols), np.int16)
    meta = np.zeros((128, 2 * nb_total), np.float32)
    b = 0
    for gi in range(n_groups):
        a, e = int(cap_off[gi]), int(cap_off[gi + 1])
        cap = e - a
        nb = cap // 128
        q = gi % NQ
        # idx: wrapped in 16 partitions, 2 copies for the queue's pair
        w16 = lc[a:e].reshape(cap // 16, 16).T  # [16, cap/16]
        o = int(idx_off[gi])
        idx_plane[32 * q:32 * q + 16, o:o + cap // 16] = w16
        idx_plane[32 * q + 16:32 * q + 32, o:o + cap // 16] = w16
        # meta: per batch (row, val) fp16 per partition
        meta[:, 2 * b:2 * (b + nb):2] = rl[a:e].reshape(nb, 128).T
        meta[:, 2 * b + 1:2 * (b + nb):2] = vv[a:e].reshape(nb, 128).T
        b += nb
    return idx_plane, meta


def _prepare(X, edge_row, edge_col, edge_val, W):
    X = np.asarray(X)
    edge_row = np.asarray(edge_row)
    edge_col = np.asarray(edge_col)
    edge_val = np.asarray(edge_val)
    W = np.asarray(W)

    caps, core, r_local, sup, chunk, gid, n_supers, chunk_sz = _plan(
        edge_row, edge_col)
    idx_off, idx_cols, batch_of, nb_total = _layout(caps, n_supers)

    key = tuple(caps.tolist())
    if key not in _PROGRAM_CACHE:
        _PROGRAM_CACHE[key] = _build_program(caps, n_supers, chunk_sz)
    nc = _PROGRAM_CACHE[key]

    X16 = np.ascontiguousarray(X.astype(np.float16))
    iota = np.tile(np.arange(SUPER_W, dtype=np.float16), (128, 1))
    ident = np.eye(128, dtype=np.float32)
    wt = np.ascontiguousarray(
        W.T.astype(np.float16).reshape(2, 128, D).transpose(1, 0, 2))

    in_maps = []
    for k in range(NCORES):
        idx_plane, meta = _pack_core(
            k, caps, core, r_local, sup, chunk, gid, edge_col, edge_val,
            chunk_sz, idx_off, idx_cols, nb_total)
        in_maps.append({"x16": X16, "idx": idx_plane, "meta": meta,
                        "iota": iota, "ident": ident, "wt": wt})
    return nc, in_maps


def _gather_out(res, b):
    rows_per_core = N_NODES // NCORES
    out = np.empty((N_NODES, D), np.float32)
    for k in range(NCORES):
        out[k * rows_per_core:(k + 1) * rows_per_core] = \
            res.results[k]["out"][:rows_per_core]
    out += np.asarray(b).astype(np.float32)[None, :]
    return out


def kernel(X, edge_row, edge_col, edge_val, W, b):
    from concourse.bass_utils import run_bass_kernel_spmd

    nc, in_maps = _prepare(X, edge_row, edge_col, edge_val, W)
    res = run_bass_kernel_spmd(nc, in_maps, core_ids=list(range(NCORES)))
    return _gather_out(res, b)


def run_traced(X, edge_row, edge_col, edge_val, W, b):
    """Run with NTFF profiling; returns BassKernelResults."""
    from concourse.bass_utils import run_bass_kernel_spmd

    nc, in_maps = _prepare(X, edge_row, edge_col, edge_val, W)
    return run_bass_kernel_spmd(nc, in_maps, core_ids=list(range(NCORES)),
                                trace=True)
